# revision 1
# baseline (speedup 1.0000x reference)
"""AttnBlock (GroupNorm + 1x1-conv spatial self-attention + residual) on 8 TRN2 cores.

Sharding: core = (batch b, pixel-quarter q). Each core computes the full
GroupNorm for its batch, then attention output rows for its 1024 pixels
(i-dim), attending over all 4096 pixels (j-dim). Inputs are host-rotated
per core so the compiled program is identical across cores (SPMD).

Algebraic folds (host side, fp64):
  - scores = hn^T (Wk^T Wq / sqrt(c)) hn  ->  one projection G = Wkq @ hn
  - bk cancels in softmax (constant along j); bq kept via bg = Wk^T bq_s
  - Wo @ Wv folded into one matrix; bo' = Wo @ bv + bo added at the end
  - softmax max-subtraction skipped (scores ~ N(0, 1/9); exp is safe)
  - 1/rowsum applied after the AV matmul (divide commutes with the
    channel-mixing projection), broadcast across partitions by a K=1
    ones outer-product matmul.
"""

import numpy as np

B, C, H, W = 2, 512, 64, 64
HW = H * W               # 4096
P = 128                  # partitions
NCK = C // P             # 4 channel chunks
QPIX = HW // 4           # 1024 pixels per core
NIB = 2                  # i-blocks of 512 per core
IBS = QPIX // NIB        # 512
NJT = HW // P            # 32 j-tiles
NSUB = HW // 512         # 8 bn_stats subgroups
EPS = 1e-6

_CACHE = {}


def _build_nc():
    import concourse.bass as bass
    import concourse.tile as tile
    from concourse import bacc, mybir
    from contextlib import ExitStack

    f32 = mybir.dt.float32
    f32r = mybir.dt.float32r
    AF = mybir.ActivationFunctionType
    OP = mybir.AluOpType

    nc = bacc.Bacc("TRN2", target_bir_lowering=False, debug=False,
                   enable_asserts=False, num_devices=8)

    x_d = nc.dram_tensor("x", [C, HW], f32r, kind="ExternalInput")
    wkqt_d = nc.dram_tensor("wkqt", [C, C], f32r, kind="ExternalInput")
    wovt_d = nc.dram_tensor("wovt", [C, C], f32r, kind="ExternalInput")
    pvec_d = nc.dram_tensor("pvec", [NCK, P, 3], f32, kind="ExternalInput")
    xt_d = nc.dram_tensor("xt", [QPIX, C], f32, kind="ExternalInput")
    out_d = nc.dram_tensor("out", [QPIX, C], f32, kind="ExternalOutput")

    # group-aggregation selectors (constant): 32 groups of 16 channels; a
    # channel chunk of 128 holds 8 whole groups.
    sel_np = np.zeros((P, 8), np.float32)
    for p in range(P):
        sel_np[p, p // 16] = 1.0 / 16.0
    selt_np = np.zeros((8, P), np.float32)
    for p in range(P):
        selt_np[p // 16, p] = 1.0
    sel_d = nc.inline_tensor(sel_np, "selc")
    selt_d = nc.inline_tensor(selt_np, "seltc")

    x_r = x_d.ap().rearrange("(c p) n -> c p n", p=P)
    out_r = out_d.ap().rearrange("(g p) o -> g p o", p=P)

    with tile.TileContext(nc) as tc, ExitStack() as ctx:
        perm = ctx.enter_context(tc.tile_pool(name="perm", bufs=1))
        gnp = ctx.enter_context(tc.tile_pool(name="gnwork", bufs=2))

        # constants
        sel_sb = perm.tile([P, 8], f32, name="sel", tag="sel")
        nc.gpsimd.dma_start(out=sel_sb, in_=sel_d.ap())
        selt_sb = perm.tile([8, P], f32, name="selt", tag="selt")
        nc.gpsimd.dma_start(out=selt_sb, in_=selt_d.ap())
        ones_sb = perm.tile([P, P], f32, name="ones", tag="ones")
        nc.vector.memset(ones_sb, 1.0)
        zscr = perm.tile([P, IBS], f32, name="zscr", tag="zscr")
        nc.vector.memset(zscr, 0.0)
        zr = perm.tile([P, IBS], f32r, name="zr", tag="zr")
        nc.vector.tensor_copy(out=zr, in_=zscr)
        eps_sb = perm.tile([8, 1], f32, name="eps", tag="eps")
        nc.vector.memset(eps_sb, EPS)

        # pvec columns per chunk: 0=gamma 1=beta 2=bg
        pvec_sb = perm.tile([P, NCK, 3], f32, name="pvec", tag="pvec")
        nc.gpsimd.dma_start(out=pvec_sb, in_=pvec_d.ap().rearrange("c p v -> p c v"))
        gamma_sb = [pvec_sb[:, ck, 0:1] for ck in range(NCK)]
        beta_sb = [pvec_sb[:, ck, 1:2] for ck in range(NCK)]
        bg_sb = [pvec_sb[:, ck, 2:3] for ck in range(NCK)]

        # x chunks (normalized in place -> hn)
        x_sb = [perm.tile([P, HW], f32r, name=f"x{ck}", tag=f"x{ck}") for ck in range(NCK)]
        for ck in range(NCK):
            for h in range(4):
                sl = slice(h * (HW // 4), (h + 1) * (HW // 4))
                nc.sync.dma_start(out=x_sb[ck][:, sl], in_=x_r[ck, :, sl])

        G_sb = [perm.tile([P, QPIX], f32r, name=f"G{ci}", tag=f"G{ci}") for ci in range(NCK)]
        vot_sb = [perm.tile([P, C], f32r, name=f"vot{p}", tag=f"vot{p}") for p in range(NJT)]
        xt_all = perm.tile([P, NIB * NCK, C], f32, name="xt_all", tag="xt_all")

        with tc.tile_pool(name="wts", bufs=1) as wts, \
             tc.tile_pool(name="psA", bufs=1, space="PSUM") as psA:
            wkqt_all = wts.tile([P, NCK, C], f32r, name="wkqt_all", tag="wkqt_all")
            nc.scalar.dma_start(out=wkqt_all,
                                in_=wkqt_d.ap().rearrange("(c p) n -> p c n", p=P))
            wovt_all = wts.tile([P, NCK, C], f32r, name="wovt_all", tag="wovt_all")
            wkqt_sb = [wkqt_all[:, ck, :] for ck in range(NCK)]
            wovt_sb = [wovt_all[:, ck, :] for ck in range(NCK)]

            nc.sync.dma_start(out=wovt_all,
                              in_=wovt_d.ap().rearrange("(c p) n -> p c n", p=P))

            # PE warmup: fp32 matmuls on zeros keep the HAM activity window
            # busy while x/weights stream in, so real matmuls run at 2.4 GHz.
            def warm_mms(n, tag):
                pw = psA.tile([P, IBS], f32, name=f"warm{tag}", tag="warm", bufs=1)
                for _ in range(n):
                    nc.tensor.matmul(pw, zr[:, 0:P], zr, start=True, stop=True)

            warm_mms(34, "w1")

            # ---- GroupNorm ----
            scale_sb, shift_sb = [], []
            for ck in range(NCK):
                stats = gnp.tile([P, NSUB, 6], f32, name="stats", tag="stats")
                for s in range(NSUB):
                    nc.vector.bn_stats(out=stats[:, s, :],
                                       in_=x_sb[ck][:, s * 512:(s + 1) * 512].bitcast(f32))
                mv = gnp.tile([P, 2], f32, name="mv", tag="mv")
                nc.vector.bn_aggr(out=mv, in_=stats)
                # cm = (mean, E[x^2]) per channel; gpsimd for early chunks
                # keeps DVE free to chase the last chunk's bn_stats
                ew = nc.vector
                cm = gnp.tile([P, 2], f32, name="cm", tag="cm")
                nc.scalar.copy(out=cm[:, 0:1], in_=mv[:, 0:1])
                nc.vector.scalar_tensor_tensor(
                    out=cm[:, 1:2], in0=mv[:, 0:1], scalar=mv[:, 0:1],
                    in1=mv[:, 1:2], op0=OP.mult, op1=OP.add)
                # aggregate to 8 groups: (gmean, gm2)
                pg8 = psA.tile([8, 2], f32, name="g8", tag="gn", bufs=1)
                nc.tensor.matmul(pg8, sel_sb, cm, start=True, stop=True)
                gm = gnp.tile([8, 2], f32, name="gm", tag="gm")
                nc.scalar.copy(out=gm, in_=pg8)
                gsq = gnp.tile([8, 1], f32, name="gsq", tag="gsq")
                ew.tensor_mul(gsq, gm[:, 0:1], gm[:, 0:1])
                gvar = gnp.tile([8, 1], f32, name="gvar", tag="gvar")
                ew.tensor_sub(gvar, gm[:, 1:2], gsq)
                gb = gnp.tile([8, 2], f32, name="gb", tag="gb")
                ew.tensor_copy(out=gb[:, 0:1], in_=gm[:, 0:1])
                nc.scalar.activation(out=gb[:, 1:2], in_=gvar, func=AF.Sqrt,
                                     bias=eps_sb, scale=1.0)
                nc.vector.reciprocal(out=gb[:, 1:2], in_=gb[:, 1:2])
                # broadcast group (mean, rstd) back to 128 channels
                pbc2 = psA.tile([P, 2], f32, name="bc2", tag="gn", bufs=1)
                nc.tensor.matmul(pbc2, selt_sb, gb, start=True, stop=True)
                scl = gnp.tile([P, 1], f32, name=f"scl{ck}", tag=f"scl{ck}", bufs=1)
                nc.vector.tensor_mul(scl, pbc2[:, 1:2], gamma_sb[ck])
                tmp = gnp.tile([P, 1], f32, name="tmp", tag="tmp")
                nc.vector.tensor_mul(tmp, pbc2[:, 0:1], scl)
                shf = gnp.tile([P, 1], f32, name=f"shf{ck}", tag=f"shf{ck}", bufs=1)
                nc.vector.tensor_sub(shf, beta_sb[ck], tmp)
                scale_sb.append(scl)
                shift_sb.append(shf)
                warm_mms(8, f"wgn{ck}")
                # hn = x * scale + shift  (in place)
                for nsl in range(4):
                    sl = slice(nsl * QPIX, (nsl + 1) * QPIX)
                    if nsl % 2 == 0:
                        nc.scalar.activation(out=x_sb[ck][:, sl],
                                             in_=x_sb[ck][:, sl].bitcast(f32),
                                             func=AF.Identity, bias=shf, scale=scl)
                    else:
                        nc.vector.tensor_scalar(
                            out=x_sb[ck][:, sl], in0=x_sb[ck][:, sl].bitcast(f32),
                            scalar1=scl, scalar2=shf,
                            op0=OP.mult, op1=OP.add)

            hn = x_sb

            # ---- G = Wkq @ hn + bg  (chunk-major so PE starts early) ----
            for ib in range(NIB):
                pgs = [psA.tile([P, IBS], f32, name=f"g{ci}", tag=f"g{ci}", bufs=1)
                       for ci in range(NCK)]
                for ckp in range(NCK):
                    for ci in range(NCK):
                        nc.tensor.matmul(
                            pgs[ci],
                            wkqt_sb[ckp][:, ci * P:(ci + 1) * P],
                            hn[ckp][:, ib * IBS:(ib + 1) * IBS],
                            start=(ckp == 0), stop=(ckp == NCK - 1))
                for ci in range(NCK):
                    nc.vector.tensor_scalar_add(
                        out=G_sb[ci][:, ib * IBS:(ib + 1) * IBS],
                        in0=pgs[ci], scalar1=bg_sb[ci])
                warm_mms(10 if ib == 0 else 22, f"w{ib + 2}")

            # ---- voT = hn^T @ Wov^T ----
            for p in range(NJT):
                pv = psA.tile([P, C], f32, name="vt", tag="vt", bufs=2)
                for ck in range(NCK):
                    nc.tensor.matmul(
                        pv,
                        hn[ck][:, p * P:(p + 1) * P],
                        wovt_sb[ck],
                        start=(ck == 0), stop=(ck == NCK - 1))
                if p % 2 == 0:
                    nc.scalar.copy(out=vot_sb[p], in_=pv)
                else:
                    nc.vector.tensor_copy(out=vot_sb[p], in_=pv)

        # residual (transposed, host-folded) — needed only in the tail;
        # late gpsimd DMA keeps it off the head's HBM bandwidth
        nc.sync.dma_start(out=xt_all, in_=xt_d.ap().rearrange("(g p) o -> p g o", p=P))

        # ---- attention ----
        with tc.tile_pool(name="att", bufs=2) as att, \
             tc.tile_pool(name="psB", bufs=1, space="PSUM") as psB:
            for ib in range(NIB):
                pavs = [psB.tile([P, C], f32, name=f"av{ok}", tag="av", bufs=5)
                        for ok in range(NCK)]
                racc = att.tile([P, IBS], f32, name="racc", tag="racc", bufs=2)

                def av_group(jt, e_t):
                    for isub in range(NCK):
                        nc.tensor.matmul(
                            pavs[isub],
                            e_t[:, isub * P:(isub + 1) * P],
                            vot_sb[jt],
                            start=(jt == 0), stop=(jt == NJT - 1),
                            skip_group_check=True)

                pend = None  # (jt, e_sb) with exp in flight; av emitted next iter
                for jt in range(NJT):
                    pe = psB.tile([P, IBS], f32, name="e", tag="e", bufs=2)
                    for ck in range(NCK):
                        nc.tensor.matmul(
                            pe,
                            hn[ck][:, jt * P:(jt + 1) * P],
                            G_sb[ck][:, ib * IBS:(ib + 1) * IBS],
                            start=(ck == 0), stop=(ck == NCK - 1))
                    if pend is not None:
                        av_group(*pend)
                    e_sb = att.tile([P, IBS], f32r, name="e_sb", tag="e_sb", bufs=3)
                    nc.scalar.activation(out=e_sb, in_=pe, func=AF.Exp)
                    if jt == 0:
                        nc.vector.tensor_copy(out=racc, in_=e_sb.bitcast(f32))
                    else:
                        nc.vector.tensor_add(racc, racc, e_sb.bitcast(f32))
                    pend = (jt, e_sb)
                av_group(*pend)
                # transposed rowsums: prT[:, s] = sum_p racc[p, s*128:(s+1)*128]
                prT = psB.tile([P, NCK], f32, name="rT", tag="rT", bufs=1)
                for s in range(NCK):
                    nc.tensor.matmul(prT[:, s:s + 1],
                                     racc[:, s * P:(s + 1) * P],
                                     ones_sb[:, 0:1],
                                     start=True, stop=True, skip_group_check=True)
                rT_sb = att.tile([P, NCK], f32, name="rT_sb", tag="rT_sb", bufs=2)
                nc.vector.reciprocal_approx_fast(out=rT_sb, in_=prT)
                for isub in range(NCK):
                    g = ib * NCK + isub
                    t = att.tile([P, C], f32, name="t_out", tag="t_out", bufs=3)
                    nc.vector.scalar_tensor_tensor(
                        out=t, in0=pavs[isub], scalar=rT_sb[:, isub:isub + 1],
                        in1=xt_all[:, g, :],
                        op0=OP.mult, op1=OP.add)
                    nc.sync.dma_start(out=out_r[g], in_=t)

    nc.compile()
    return nc


def _get_nc():
    if "nc" not in _CACHE:
        _CACHE["nc"] = _build_nc()
    return _CACHE["nc"]


def make_in_maps(**inputs):
    x = np.asarray(inputs["x"], np.float64).reshape(B, C, HW)
    gamma = np.asarray(inputs["gamma"], np.float64)
    beta = np.asarray(inputs["beta"], np.float64)
    wq = np.asarray(inputs["wq"], np.float64)
    bq = np.asarray(inputs["bq"], np.float64)
    wk = np.asarray(inputs["wk"], np.float64)
    wv = np.asarray(inputs["wv"], np.float64)
    bv = np.asarray(inputs["bv"], np.float64)
    wo = np.asarray(inputs["wo"], np.float64)
    bo = np.asarray(inputs["bo"], np.float64)
    cs = 1.0 / np.sqrt(C)

    wkqt = ((wq.T @ wk) * cs).astype(np.float32)            # [ci', ci]
    bg = wk.T @ (bq * cs)
    wovt = (wv.T @ wo.T).astype(np.float32)                 # [ci, o]
    addc = (wo @ bv + bo).astype(np.float32)
    pvec = np.ascontiguousarray(
        np.stack([gamma.reshape(NCK, P), beta.reshape(NCK, P),
                  bg.reshape(NCK, P)], axis=2).astype(np.float32))

    in_maps = []
    for core in range(8):
        b, q = divmod(core, 4)
        xb = np.roll(x[b], -q * QPIX, axis=1).astype(np.float32)
        xt = np.ascontiguousarray(xb[:, :QPIX].T + addc[None, :])
        in_maps.append({
            "x": np.ascontiguousarray(xb),
            "wkqt": wkqt, "wovt": wovt, "pvec": pvec, "xt": xt,
        })
    return in_maps


def assemble(results):
    out = np.empty((B, C, HW), np.float32)
    for core in range(8):
        b, q = divmod(core, 4)
        out[b][:, q * QPIX:(q + 1) * QPIX] = results[core]["out"].T
    return out.reshape(B, C, H, W)


def kernel(**inputs):
    from concourse.bass_utils import run_bass_kernel_spmd
    nc = _get_nc()
    in_maps = make_in_maps(**inputs)
    res = run_bass_kernel_spmd(nc, in_maps, core_ids=list(range(8)))
    return assemble(res.results)

